# revision 1
# baseline (speedup 1.0000x reference)
"""Deformable-conv Trainium2 kernel (8-core SPMD, bass/Tile)."""
"""Patch TileContext tail-drain: this walrus build rejects >2 sync waits per instruction."""
import sys
for _p in ("/opt/trn_rl_repo", "/root/.axon_site/_ro/trn_rl_repo"):
    import os as _os
    if _os.path.isdir(_p) and _p not in sys.path:
        sys.path.insert(0, _p)
import bass_rust
import concourse.tile as tile
from concourse.vector_clock import ScopedClock

_MAX_WAITS = 1

def _patched_drain_and_barrier(self, tick_clock, wait_clock):
    nc = self.nc
    drain_inst = nc.sync.drain()
    wait_clock.add_sem_waits(drain_inst.ins, ScopedClock({None: tick_clock.global_clock}))
    raw = drain_inst.ins
    si = raw.sync_info
    waits = list(si.on_wait or []) if si is not None else []
    if len(waits) > _MAX_WAITS:
        si.on_wait = waits[:_MAX_WAITS]
        rest = waits[_MAX_WAITS:]
        for i in range(0, len(rest), _MAX_WAITS):
            extra = nc.sync.drain()
            eraw = extra.ins
            chunk = rest[i:i + _MAX_WAITS]
            if eraw.sync_info is None:
                eraw.sync_info = bass_rust.SyncInfo(on_wait=chunk, on_update=[])
            else:
                eraw.sync_info.on_wait = chunk

    nc.all_engine_barrier()
    assert self.sems is not None
    popped = nc._tile_sem_poison_stack.pop()
    assert popped is self._sem_poison
    nc.clear_and_free_semaphores(list(self.sems.allocated().values()))
    nc.all_engine_barrier()

tile.TileContext._drain_and_barrier = _patched_drain_and_barrier


def split_multi_waits(nc, max_waits=1):
    """Walrus in this build rejects >1 sync wait per instruction: hoist extras
    onto NOPs inserted just before, on the same engine."""
    import concourse.mybir as mybir
    for f in nc.m.functions:
        for bb in f.blocks:
            insts = bb.instructions
            i = 0
            while i < len(insts):
                inst = insts[i]
                si = inst.sync_info
                if si is not None and si.on_wait and len(si.on_wait) > max_waits:
                    waits = list(si.on_wait)
                    si.on_wait = waits[-max_waits:]
                    extra = waits[:-max_waits]
                    nops = []
                    for j in range(0, len(extra), max_waits):
                        n = mybir.InstNoOp(name=f"{inst.name}-w{j}", ins=[], outs=[])
                        n.engine = inst.engine
                        n.sync_info = bass_rust.SyncInfo(
                            on_wait=extra[j:j + max_waits], on_update=[])
                        nops.append(n)
                    for k, n in enumerate(nops):
                        insts.insert(i + k, n)
                        try:
                            nc.register_instruction(n, overwrite=True)
                        except Exception:
                            pass
                    i += len(nops)
                i += 1


# Enable DynamicDMA lowering in walrus (indirect/offset-table DMAs).
import concourse.bass_utils as _bu
_orig_gwa = _bu.get_walrus_args

def _gwa_dyn(*a, **k):
    return _orig_gwa(*a, **k) + [
        "--dge-levels=io,spill_reload,scalar_dynamic_offset,vector_dynamic_offsets",
    ]

if _bu.get_walrus_args is not _gwa_dyn:
    _bu.get_walrus_args = _gwa_dyn


"""Deformable conv TRN2 kernel: device-builder + host prep.

Per-core shard: core = (b, half): b = core//2, h0 = 60*(core%2).
Device pipeline per output row hh in [0,60):
  1. offset conv on PE: 25 tap-matmuls lhsT=volT-slice[32c,120w] x rhs=w_offs[32,100] -> PSUM [120w,100ch]
     ch layout: [0:50) = off_y slots (g*25+k), [50:100) = off_x
  2. positions/idx/weights on DVE in [w-part, free] layout
  3. indirect DMA gather: 100 tokens/row-of-pixels (50 slots x 2 y-rows), 128B each, from bf16 volq
  4. bilinear combine on DVE (7 TT ops, bf16, weight free-broadcast over c)
  5. S-transpose (PE) -> einsum chunk matmuls -> PSUM [120,64] -> bias -> DRAM
"""
import numpy as np
import concourse.bass as bass
import concourse.bacc as bacc
import concourse.mybir as mybir
import concourse.tile as tile

F32 = mybir.dt.float32
BF16 = mybir.dt.bfloat16
I32 = mybir.dt.int32
Alu = mybir.AluOpType

H = 128; W = 128; C = 32
K = 25; G = 2; Fh = 5; Fw = 5; OW = 120
NCH = 100          # offset channels (y-block 50 | x-block 50)
NS = 50            # (g,k) slots
HPC = 60           # output rows per core
CONV_ROWS = HPC + 8  # volume rows the conv needs


def host_prep(volume, w_off, b_off, w_dcn, b_dcn, n_cores=8, hpc=HPC):
    """Per-core input maps. Pure layout permutation / replication marshalling."""
    # permuted w_off: ch' = axis*50 + g*25 + k  <-  ch = k*4 + axis*2 + g
    chp = np.empty(NCH, np.int64)
    for axis in range(2):
        for g in range(G):
            for k in range(K):
                chp[axis * 50 + g * 25 + k] = k * (2 * G) + axis * G + g
    w_offT = np.ascontiguousarray(
        w_off.reshape(Fh * Fw, C, NCH)[:, :, chp]).astype(np.float32)  # [25, 32, 100]

    kys = np.arange(-4, 5, 2, np.float32)
    kxs = np.arange(-4, 5, 2, np.float32)
    kus, kvs = np.meshgrid(kxs, kys)
    kdy = kvs.reshape(-1); kdx = kus.reshape(-1)          # tap k = ky*5 + kx
    posk = np.empty(NCH, np.float32)
    for g in range(G):
        posk[g * 25:(g + 1) * 25] = kdy + 4.0
        posk[50 + g * 25:50 + (g + 1) * 25] = kdx + 4.0
    posadd = np.tile(posk[None, :], (128, 1)).astype(np.float32)
    b_off_t = np.tile(b_off[chp][None, :], (128, 1)).astype(np.float32)

    iota_w = np.arange(128, dtype=np.float32)[:, None].copy()
    ident = np.eye(128, dtype=np.float32)

    wr = w_dcn.reshape(K, C, G, 32)
    wdT = np.zeros((128, 2 * 7, 32), np.float32)
    for g in range(G):
        for j in range(7):
            for i, k in enumerate(range(4 * j, min(4 * j + 4, K))):
                wdT[32 * i:32 * (i + 1), g * 7 + j, :] = wr[k, :, g, :]
    b_dcn_t = np.tile(b_dcn[None, :], (128, 1)).astype(np.float32)

    in_maps = []
    for core in range(n_cores):
        b = core // 2
        h0 = HPC * (core % 2)
        vol_full = np.ascontiguousarray(volume[b].reshape(H * W, C)).astype(np.float32)
        vol_conv = np.ascontiguousarray(
            volume[b, h0:h0 + CONV_ROWS].reshape(CONV_ROWS * W, C)).astype(np.float32)
        h0v = np.full((128, 1), float(h0), np.float32)
        in_maps.append({
            "vol_full": vol_full, "vol_conv": vol_conv,
            "w_offT": w_offT, "posadd": posadd, "b_off_t": b_off_t,
            "iota_w": iota_w, "ident_f": ident,
            "wdT": wdT, "b_dcn_t": b_dcn_t,
            "h0v": h0v,
        })
    return in_maps


def build_nc(hpc=HPC, debug_taps=False):
    nc = bacc.Bacc("TRN2", target_bir_lowering=False, debug=False)
    vol_full = nc.dram_tensor("vol_full", [H * W, C], F32, kind="ExternalInput")
    vol_conv = nc.dram_tensor("vol_conv", [CONV_ROWS * W, C], F32, kind="ExternalInput")
    w_offT = nc.dram_tensor("w_offT", [K, C, NCH], F32, kind="ExternalInput")
    posadd = nc.dram_tensor("posadd", [128, NCH], F32, kind="ExternalInput")
    b_off_t = nc.dram_tensor("b_off_t", [128, NCH], F32, kind="ExternalInput")
    iota_w = nc.dram_tensor("iota_w", [128, 1], F32, kind="ExternalInput")
    ident_f = nc.dram_tensor("ident_f", [128, 128], F32, kind="ExternalInput")

    wdT = nc.dram_tensor("wdT", [128, 14, 32], F32, kind="ExternalInput")
    b_dcn_t = nc.dram_tensor("b_dcn_t", [128, 64], F32, kind="ExternalInput")
    h0v = nc.dram_tensor("h0v", [128, 1], F32, kind="ExternalInput")
    out = nc.dram_tensor("out", [hpc, OW, 64], F32, kind="ExternalOutput")
    if debug_taps:
        dbg_po = nc.dram_tensor("dbg_po", [OW, NCH], F32, kind="ExternalOutput")
        dbg_base = nc.dram_tensor("dbg_base", [OW, NCH], F32, kind="ExternalOutput")
        dbg_wgt = nc.dram_tensor("dbg_wgt", [OW, NCH], F32, kind="ExternalOutput")
        dbg_idx = nc.dram_tensor("dbg_idx", [OW, NCH], I32, kind="ExternalOutput")
    # gather source: full 2x2 patch per pixel [v00|v01|v10|v11], 512B fp32 rows, +132 headroom
    volq3 = nc.dram_tensor("volq3", [H * W + 136, 4 * C], F32)

    with tile.TileContext(nc) as tc:
        with (
            tc.tile_pool(name="stage", bufs=2) as stp,
            tc.tile_pool(name="res", bufs=1) as resp,
            tc.tile_pool(name="psA", bufs=2, space="PSUM") as psA,   # conv out + staging transposes
            tc.tile_pool(name="psB", bufs=3, space="PSUM") as psB,   # einsum transposes
            tc.tile_pool(name="psC", bufs=1, space="PSUM") as psC,   # einsum out
            tc.tile_pool(name="work", bufs=2) as wkp,
            tc.tile_pool(name="gtp", bufs=1) as gtp,
        ):
            # ---------- resident tiles ----------
            volT = resp.tile([C, CONV_ROWS * W], BF16)   # [c, (y,x)] conv source, core-relative rows
            w_offs = resp.tile([C, K * NCH], BF16)
            wds = resp.tile([128, 14 * 32], F32)
            pos_c = resp.tile([128, NCH], F32)
            iw = resp.tile([128, 1], F32)
            h0t = resp.tile([128, 1], F32)
            idn = resp.tile([128, 128], BF16)
            idnf = resp.tile([128, 128], F32)

            bdc = resp.tile([128, 64], F32)

            # ---------- staging ----------
            nc.sync.dma_start(iw[:], iota_w[:])
            nc.sync.dma_start(h0t[:], h0v[:])
            nc.sync.dma_start(bdc[:], b_dcn_t[:])
            nc.sync.dma_start(idnf[:], ident_f[:])

            nc.vector.tensor_copy(idn[:], idnf[:])
            pa = stp.tile([128, NCH], F32, tag="pa")
            nc.sync.dma_start(pa[:], posadd[:])
            pb = stp.tile([128, NCH], F32, tag="pb")
            nc.sync.dma_start(pb[:], b_off_t[:])
            nc.vector.tensor_tensor(out=pos_c[:], in0=pa[:], in1=pb[:], op=Alu.add)
            wof = stp.tile([C, K * NCH], F32, tag="wof")
            # w_offT dram [K, C, NCH] -> SBUF [C, (k, ch)]
            nc.sync.dma_start(wof[:], bass.AP(w_offT[:].tensor, 0,
                                              [[NCH, C], [C * NCH, K], [1, NCH]]))
            nc.vector.tensor_copy(w_offs[:], wof[:])
            nc.sync.dma_start(wds[:], wdT[:].rearrange("p a b -> p (a b)"))

            zt = stp.tile([128, 128], F32, tag="zt")
            nc.vector.memset(zt[:], 0.0)
            nc.sync.dma_start(bass.AP(volq3[:].tensor, 0, [[128, 128], [1, 128]]), zt[:])
            nc.sync.dma_start(bass.AP(volq3[:].tensor, 128 * 128, [[128, 4], [1, 128]]), zt[0:4, :])
            nc.sync.dma_start(bass.AP(volq3[:].tensor, 16387 * 128, [[128, 128], [1, 128]]), zt[:])
            nc.sync.dma_start(bass.AP(volq3[:].tensor, 16515 * 128, [[128, 5], [1, 128]]), zt[0:5, :])
            # volq3[r + 132 - dy*128 - dx, (dy*2+dx)*32 : +32] = vol[r]
            for j in range(16):
                ch = stp.tile([128, 8 * 32], F32, tag="stg_in")
                nc.sync.dma_start(ch[:], bass.AP(vol_full[:].tensor, j * 128 * 8 * 32,
                                                 [[8 * 32, 128], [1, 8 * 32]]))
                for sft in range(4):
                    dy, dx = sft >> 1, sft & 1
                    nc.sync.dma_start(
                        bass.AP(volq3[:].tensor,
                                (j * 1024 + 132 - dy * 128 - dx) * 128 + sft * 32,
                                [[8 * 128, 128], [128, 8], [1, 32]]),
                        ch[:].rearrange("p (r c) -> p r c", c=32))

            # volT: load vol_conv as [x-part, (y, c)], cast, then per-y PE-transpose [128x,32c]->[32c,128x]
            vcx = resp.tile([W, CONV_ROWS * C], BF16)
            vcf = stp.tile([W, CONV_ROWS * C], F32, tag="vcf")
            nc.sync.dma_start(vcf[:], bass.AP(vol_conv[:].tensor, 0,
                                              [[C, W], [W * C, CONV_ROWS], [1, C]]))
            nc.vector.tensor_copy(vcx[:], vcf[:])
            for y4 in range(0, CONV_ROWS, 4):
                pt = psA.tile([C, 4 * W], BF16, space="PSUM", tag="conv")
                for i in range(4):
                    y = y4 + i
                    nc.tensor.transpose(out=pt[:, i * W:(i + 1) * W],
                                        in_=vcx[:, y * C:(y + 1) * C], identity=idn[:])
                nc.scalar.copy(volT[:, y4 * W:(y4 + 4) * W], pt[:])

            # ---------- per output row ----------
            for hh in range(hpc):
                # 1. offset conv
                cps = psA.tile([OW, NCH], F32, space="PSUM", tag="conv")
                for ky in range(Fh):
                    for kx in range(Fw):
                        k = ky * 5 + kx
                        o = (hh + 2 * ky) * W + 2 * kx
                        nc.tensor.matmul(out=cps[:], lhsT=volT[:, o:o + OW],
                                         rhs=w_offs[:, k * NCH:(k + 1) * NCH],
                                         start=(k == 0), stop=(k == K - 1))
                # 2. positions
                po = wkp.tile([OW, NCH], F32, tag="po")
                nc.vector.tensor_tensor(out=po[:], in0=cps[:], in1=pos_c[0:OW, :], op=Alu.add)
                nc.vector.tensor_scalar(out=po[:, 0:50], in0=po[:, 0:50], scalar1=h0t[0:OW, :],
                                        scalar2=float(hh), op0=Alu.add, op1=Alu.add)
                nc.vector.tensor_scalar(out=po[:, 50:100], in0=po[:, 50:100], scalar1=iw[0:OW, :],
                                        scalar2=None, op0=Alu.add)
                nc.vector.tensor_scalar(out=po[:], in0=po[:], scalar1=0.0, scalar2=127.0,
                                        op0=Alu.max, op1=Alu.min)
                base = wkp.tile([OW, NCH], F32, tag="base")
                nc.vector.tensor_scalar(out=base[:], in0=po[:], scalar1=-0.5,
                                        scalar2=float(3 * 2**22), op0=Alu.add, op1=Alu.add)
                nc.vector.tensor_scalar(out=base[:], in0=base[:], scalar1=-float(3 * 2**22),
                                        scalar2=None, op0=Alu.add)
                nc.vector.tensor_scalar(out=base[:], in0=base[:], scalar1=126.0, scalar2=None, op0=Alu.min)
                wgt = wkp.tile([OW, NCH], F32, tag="wgt")
                nc.vector.tensor_tensor(out=wgt[:], in0=po[:], in1=base[:], op=Alu.subtract)
                if debug_taps and hh == 0:
                    nc.sync.dma_start(dbg_po[:], po[:])
                    nc.sync.dma_start(dbg_base[:], base[:])
                    nc.sync.dma_start(dbg_wgt[:], wgt[:])
                # 3. gather indices: cols 0:50 = y0*128+x0 ; cols 50:100 = +128
                idxf = wkp.tile([128, NCH], F32, tag="idxf")
                nc.vector.memset(idxf[96:128, :], 0.0)
                nc.vector.tensor_scalar(out=idxf[0:OW, 0:50], in0=base[:, 0:50], scalar1=128.0,
                                        scalar2=None, op0=Alu.mult)
                nc.vector.tensor_tensor(out=idxf[0:OW, 0:50], in0=idxf[0:OW, 0:50],
                                        in1=base[:, 50:100], op=Alu.add)
                nc.vector.tensor_scalar(out=idxf[0:OW, 0:50], in0=idxf[0:OW, 0:50], scalar1=132.0,
                                        scalar2=None, op0=Alu.add)
                idxi = wkp.tile([128, NS], I32, tag="idxi")
                nc.vector.tensor_copy(idxi[:], idxf[:, 0:NS])
                if debug_taps and hh == 0:
                    nc.sync.dma_start(dbg_idx[:], idxi[:])
                # 4. bilinear weights bf16 [120, 200] = w00|w01|w10|w11
                wq = wkp.tile([OW, 4 * NS], F32, tag="wq")
                omw = wkp.tile([OW, NCH], F32, tag="omw")
                nc.vector.tensor_scalar(out=omw[:], in0=wgt[:], scalar1=-1.0, scalar2=1.0,
                                        op0=Alu.mult, op1=Alu.add)
                nc.vector.tensor_tensor(out=wq[:, 0:50], in0=omw[:, 0:50], in1=omw[:, 50:100], op=Alu.mult)
                nc.vector.tensor_tensor(out=wq[:, 50:100], in0=omw[:, 0:50], in1=wgt[:, 50:100], op=Alu.mult)
                nc.vector.tensor_tensor(out=wq[:, 100:150], in0=wgt[:, 0:50], in1=omw[:, 50:100], op=Alu.mult)
                nc.vector.tensor_tensor(out=wq[:, 150:200], in0=wgt[:, 0:50], in1=wgt[:, 50:100], op=Alu.mult)
                # x2-duplicated bf16 weights: wqb2[w, j, r] = wq[w, j] for r in {0,1}.
                # Lets the combine mults read weights with a packed [1,2] last dim
                # (2x_1P mode) without a full c-replication.
                wqb2 = wkp.tile([OW, 4 * NS, 2], F32, tag="wqb2")
                nc.vector.tensor_copy(
                    wqb2[:], wq[:].unsqueeze(2).broadcast_to([OW, 4 * NS, 2]))
                # 5. gather: per slot, one token per partition ([P,1] offsets;
                # offset unit = in-row SIZE = 512B = 1 pixel of the windowed layout)
                gt = gtp.tile([128, NS, 4 * C], F32, tag="gt")
                vol_view = bass.AP(volq3[:].tensor, 0, [[128, H * W + 136], [1, 128]])
                for sl in range(NS):
                    nc.gpsimd.indirect_dma_start(
                        out=gt[:, sl, :], out_offset=None, in_=vol_view,
                        in_offset=bass.IndirectOffsetOnAxis(ap=idxi[:, sl:sl + 1], axis=0))
                # 6. combine: per y-row r: T_r = v_r0*w_r0 + v_r1*w_r1 on DVE (all packed, 2x);
                #    the y-sum happens in PSUM via accumulating transposes.
                T0 = wkp.tile([OW, NS * C], F32, tag="T0")
                T1 = wkp.tile([OW, NS * C], F32, tag="T1")
                tm0 = wkp.tile([OW, NS * C], F32, tag="tm0")
                tm1 = wkp.tile([OW, NS * C], F32, tag="tm1")

                def gv(row, px):
                    a = gt[:]
                    return bass.AP(a.tensor, a.offset + (row * 2 + px) * C,
                                   [[a.ap[0][0], OW], [4 * C, NS], [1, C]])

                def sv(t):
                    a = t[:]
                    return bass.AP(a.tensor, a.offset, [a.ap[0], [C, NS], [1, C]])

                def wb(col):
                    # [w, slot, c] view of the x2-duplicated weights: packed last dim [1,2]
                    a = wqb2[:]
                    return bass.AP(a.tensor, a.offset + col * NS * 2,
                                   [a.ap[0], [2, NS], [0, C // 2], [1, 2]])

                nc.vector.tensor_tensor(out=sv(tm0), in0=gv(0, 0), in1=wb(0), op=Alu.mult)
                nc.vector.tensor_tensor(out=sv(tm1), in0=gv(0, 1), in1=wb(1), op=Alu.mult)
                nc.vector.tensor_tensor(out=sv(T0), in0=sv(tm0), in1=sv(tm1), op=Alu.add)
                nc.vector.tensor_tensor(out=sv(tm0), in0=gv(1, 0), in1=wb(2), op=Alu.mult)
                nc.vector.tensor_tensor(out=sv(tm1), in0=gv(1, 1), in1=wb(3), op=Alu.mult)
                nc.vector.tensor_tensor(out=sv(T1), in0=sv(tm0), in1=sv(tm1), op=Alu.add)
                # 7. einsum: accumulate transpose(T0)+transpose(T1) in PSUM (y-sum on PE),
                #    4 chunks batched per PSUM tile, one ACT copy per batch.
                ops0 = psC.tile([OW, 32], F32, space="PSUM", tag="out0")
                ops1 = psC.tile([OW, 32], F32, space="PSUM", tag="out1")
                opsg = [ops0, ops1]
                chunks = ([(g, j) for g in range(G) for j in range(6)]
                          + [(0, 6), (1, 6)])  # wd=32 chunks in their own last batch
                for batch0 in range(0, 14, 4):
                    bchunks = chunks[batch0:batch0 + 4]
                    nb = len(bchunks)
                    wd = 128 if batch0 < 12 else 32
                    tps = psB.tile([128, nb * OW], F32, space="PSUM", tag="tsp")
                    for i, (g, j) in enumerate(bchunks):
                        c0 = g * 800 + j * 128
                        for r, T in enumerate((T0, T1)):
                            nc.tensor.matmul(out=tps[0:wd, i * OW:(i + 1) * OW],
                                             lhsT=T[:, c0:c0 + wd],
                                             rhs=idnf[0:OW, 0:OW], is_transpose=True,
                                             start=(r == 0), stop=(r == 1))
                    tss = wkp.tile([128, nb * OW], F32, tag="tss")
                    nc.scalar.copy(tss[0:wd, :], tps[0:wd, :])
                    for i, (g, j) in enumerate(bchunks):
                        nc.tensor.matmul(out=opsg[g][:],
                                         lhsT=tss[0:wd, i * OW:(i + 1) * OW],
                                         rhs=wds[0:wd, (g * 7 + j) * 32:(g * 7 + j + 1) * 32],
                                         start=(j == 0), stop=(j == 6))
                # 8. bias + out
                ot = wkp.tile([OW, 64], F32, tag="ot")
                for g in range(G):
                    nc.vector.tensor_tensor(out=ot[:, g * 32:(g + 1) * 32], in0=opsg[g][:],
                                            in1=bdc[0:OW, g * 32:(g + 1) * 32], op=Alu.add)
                nc.sync.dma_start(out[hh], ot[:])
    nc.compile()
    split_multi_waits(nc)
    return nc


_NC_CACHE = {}


def kernel(volume, w_off, b_off, w_dcn, b_dcn):
    """Deformable conv on 8 trn2 cores: full inputs in, full output out."""
    import numpy as _np
    from concourse.bass_utils import run_bass_kernel_spmd
    volume = _np.asarray(volume, _np.float32)
    w_off = _np.asarray(w_off, _np.float32)
    b_off = _np.asarray(b_off, _np.float32)
    w_dcn = _np.asarray(w_dcn, _np.float32)
    b_dcn = _np.asarray(b_dcn, _np.float32)
    in_maps = host_prep(volume, w_off, b_off, w_dcn, b_dcn)
    if "nc" not in _NC_CACHE:
        _NC_CACHE["nc"] = build_nc(hpc=HPC)
    nc = _NC_CACHE["nc"]
    res = run_bass_kernel_spmd(nc, in_maps, list(range(8)))
    out = _np.empty((4, 120, 120, 64), _np.float32)
    for core in range(8):
        b = core // 2
        h0 = HPC * (core % 2)
        out[b, h0:h0 + HPC] = res.results[core]["out"]
    return out

